# revision 38
# baseline (speedup 1.0000x reference)
# Trainium2 Bass kernel for MemoryAttention (B=2, L=2048, D=1024, H=16, HD=64,
# CTX=2048, PERS=256 -> S=4352), sharded over 8 NeuronCores as
# (batch, head-group-of-4). Self-contained: hardcodes all shapes.
#
# v4: row-tiled PE (64x128 tiling) for QK and PV. Each 64-row tile of the
# PE array computes one head's contraction (head-dim=64) concurrently with
# the other head on the other tile, so a QK pair (both heads, one s-tile)
# streams in ~512 cycles instead of 2x512. PV likewise splits each s-tile's
# 128 contraction rows across the two array tiles (4 MMs in 2 concurrent
# pairs), keeping everything in one tiling mode (no PE drains between QK
# and PV). PV's two accumulator banks are written in a fixed
# (T0->pv_e, T8->pv_o), (T8->pv_e, T0->pv_o) order so same-bank writers
# are never concurrent (MM starts are pc-monotone, per-tile FIFO).
#
# Per-core layout:
#   extT  [D, S]    bf16  (ext = [ctx; pers; x_b], transposed on host)
#   QT_m  [128, L]  = (x Wq + bq)^T   m-tile rows: head 2m dims 0-63,
#                     head 2m+1 dims 64-127
#   KT_m  [128, S]  = (ext Wk + bk)^T same row layout as QT (head pair)
#   V     [S, 4*65] = ext Wv  (+ ones column per head for softmax sums)
#   sc    PSUM ring: per s-tile, even-head scores then odd-head scores
#   e     = exp(0.125 * sc)  (no max-subtraction: |scores|<=~9 here, exp is
#                     exact-safe in f32)
#   pv_h  [65, 1024] = sum_s V_aug[s]^T E[s]: rows 0..63 ctx^T, row 64 sums
#   ctxT_m[128, L]  = pv pair * (1/sums)  (bf16)
#   out   [L, D]    = sum_m ctxT_m^T Wo_m (+ host adds bo + bv@Wo)
import os
import numpy as np
import ml_dtypes

D = 1024
HD = 64
B = 2
L = 2048
CTX = 2048
PERS = 256
S = CTX + PERS + L          # 4352
NCORES = 8
HPC = 4                     # heads per core
HDPC = HPC * HD             # 256
KT = D // 128               # 8 k-tiles
NST = S // 128              # 34 s-tiles
SCALE = 1.0 / np.sqrt(HD)   # 0.125

BF16 = ml_dtypes.bfloat16

_BUILT = {}
LAST_EXEC_TIME_NS = None


def _split_multiwaits(nc):
    """This walrus build accepts at most ONE sync-wait command per engine
    instruction (2 for EventSemaphore). Tile emits instructions with several
    waits (and a closing drain with one wait per live proc). Legalize by
    hoisting extra waits onto same-engine NoOps inserted just before the
    instruction — strictly more conservative ordering, so still correct."""
    import concourse.mybir as mybir

    ctr = [0]
    for fn in nc.m.functions:
        for bb in fn.blocks:
            changed = False
            new = []
            for inst in bb.instructions:
                si = inst.sync_info
                limit = 2 if isinstance(inst, mybir.InstEventSemaphore) else 1
                if si is not None and si.on_wait and len(si.on_wait) > limit:
                    waits = list(si.on_wait)
                    for w in waits[:-limit]:
                        ctr[0] += 1
                        nop = mybir.InstNoOp(
                            name=f"wsplit_{ctr[0]}",
                            engine=inst.engine,
                            sync_info=mybir.SyncInfo(
                                on_wait=[w], on_update=[]),
                        )
                        new.append(nop)
                    si.on_wait = waits[-limit:]
                    changed = True
                new.append(inst)
            if changed:
                bb.instructions = new
    return ctr[0]


def _drop_self_waits(nc):
    """Drop sem waits where a compute-engine instruction waits on its OWN
    engine's completion sem (WAW/WAR vs an earlier same-engine op): engines
    execute their queue in order, one op at a time, so these are satisfied
    by construction. Not applied to SP/DMA (async HWDGE queues) or
    barrier/drain instructions."""
    import concourse.mybir as mybir

    eng_sem = {
        "EngineType.PE": "PE_",
        "EngineType.Activation": "Activation_",
        "EngineType.DVE": "DVE_",
        "EngineType.Pool": "Pool_",
    }
    dropped = 0
    for fn in nc.m.functions:
        for bb in fn.blocks:
            for inst in bb.instructions:
                nm = type(inst).__name__
                if nm in ("InstDrain", "InstEventSemaphore", "InstNoOp",
                          "InstDMACopy"):
                    continue
                pref = eng_sem.get(str(inst.engine))
                si = inst.sync_info
                if pref is None or si is None or not si.on_wait:
                    continue
                kept = [w for w in si.on_wait
                        if not (w.ant_name or "").startswith(pref)]
                if len(kept) != len(si.on_wait):
                    dropped += len(si.on_wait) - len(kept)
                    si.on_wait = kept
    return dropped


def _dedupe_ldweights(nc):
    """Tile lowering emits one InstLdweights per InstMatmult even when
    consecutive matmuls use the identical stationary operand. Drop the
    redundant reloads (same weights AP + tile_position, no sem updates,
    nothing but matmuls/ldweights in between on PE)."""
    import concourse.mybir as mybir

    def ldw_key(inst):
        try:
            ap = inst.ins[0].bass_ap
            return (str(ap.tensor.name), ap.offset, str(ap.ap),
                    str(getattr(inst, "tile_position", None)),
                    str(getattr(inst, "perf_mode", None)),
                    str(getattr(inst, "is_transpose", None)))
        except Exception:
            return None

    dropped = 0
    for fn in nc.m.functions:
        for bb in fn.blocks:
            last_key = None
            new = []
            for inst in bb.instructions:
                if str(inst.engine) != "EngineType.PE":
                    new.append(inst)
                    continue
                nm = type(inst).__name__
                if nm == "InstLdweights":
                    si = inst.sync_info
                    has_upd = bool(si and si.on_update)
                    k = ldw_key(inst)
                    if k is not None and k == last_key and not has_upd:
                        if si and si.on_wait:
                            nop = mybir.InstNoOp(
                                name=f"{inst.name}_ldwkeep",
                                engine=inst.engine,
                                sync_info=mybir.SyncInfo(
                                    on_wait=list(si.on_wait), on_update=[]),
                            )
                            new.append(nop)
                        dropped += 1
                        continue
                    last_key = k
                    new.append(inst)
                elif nm == "InstMatmult":
                    new.append(inst)
                else:
                    last_key = None
                    new.append(inst)
            bb.instructions = new
    return dropped


def _build():
    if 0 in _BUILT:
        return _BUILT[0]

    from contextlib import ExitStack as _ES

    import concourse.bass as bass
    import concourse.mybir as mybir
    import concourse.tile as tile

    f32 = mybir.dt.float32
    bf16 = mybir.dt.bfloat16
    AF = mybir.ActivationFunctionType

    nc = bass.Bass(trn_type="TRN2")
    extT_d = nc.dram_tensor("extT", [D, S], bf16, kind="ExternalInput")
    wq_d = nc.dram_tensor("wq", [D, HDPC], bf16, kind="ExternalInput")
    wk_d = nc.dram_tensor("wk", [D, HDPC], bf16, kind="ExternalInput")
    wv_d = nc.dram_tensor("wv", [D, HDPC], bf16, kind="ExternalInput")
    wo_d = nc.dram_tensor("wo", [HDPC, D], bf16, kind="ExternalInput")
    bq_d = nc.dram_tensor("bq", [HDPC], f32, kind="ExternalInput")
    bk_d = nc.dram_tensor("bk", [HDPC], f32, kind="ExternalInput")
    out_d = nc.dram_tensor("out", [L, D], f32, kind="ExternalOutput")
    # scratch for bouncing [1,512] softmax sums into a [128,4] layout
    sums_d = nc.dram_tensor("sums_scratch", [16, 512], f32, kind="Internal")

    with tile.TileContext(nc) as tc, _ES() as es:
        singles = es.enter_context(tc.tile_pool(name="singles", bufs=1))
        # pv pool (2 banks) lives for the whole kernel; pass-scoped pools
        # take the other 6.
        pvp = es.enter_context(tc.tile_pool(name="pvp", bufs=1, space="PSUM"))
        esb = es.enter_context(tc.tile_pool(name="esb", bufs=5))
        nrm = es.enter_context(tc.tile_pool(name="nrm", bufs=3))
        outp = es.enter_context(tc.tile_pool(name="outsb", bufs=4))

        # -------- inputs: staged DMA with minimal gating prefix --------
        # Phase A: what the very first Q-proj + K-proj + first QK need.
        # Phase B: ctx cols for K/V-proj start.  Phase C: ctx/pers stream
        # for the kproj/vproj pacing.  Phase D: x cols for later Q-proj
        # chunks.  Queues round-robin sync/gpsimd/vector (+scalar early,
        # before the first activation needs the scalar queue).
        x0 = CTX + PERS
        qrot = [0]
        early = [nc.sync, nc.scalar, nc.gpsimd]
        late = [nc.sync, nc.gpsimd]

        def dma(out, in_, qs):
            q = qs[qrot[0] % len(qs)]
            qrot[0] += 1
            q.dma_start(out=out, in_=in_)

        def load_wfused(dram, name, qs):
            # all 8 k-tiles of a [D, HDPC] weight side by side in one
            # [128, 8*256] tile; two 3d-AP DMAs (4 k-tiles each) so the
            # byte load balances across the trigger queues
            t = singles.tile([128, KT * HDPC], bf16, tag=name, name=name)
            tv = t.rearrange("p (k c) -> p k c", k=KT)
            dv = dram.rearrange("(k p) c -> p k c", p=128)
            dma(tv[:, 0:4, :], dv[:, 0:4, :], qs)
            dma(tv[:, 4:8, :], dv[:, 4:8, :], qs)
            return t

        def load_bias(dram, name, qs):
            t = singles.tile([128, 2], f32, tag=name, name=name)
            for m in range(2):
                dma(t[:, m:m + 1],
                    dram[m * 128:(m + 1) * 128].rearrange(
                        "(p o) -> p o", o=1), qs)
            return t

        extT = [singles.tile([128, S], bf16, tag=f"extT{k}",
                             name=f"extT{k}") for k in range(KT)]

        def load_ext(c0, c1, qs):
            for k in range(KT):
                dma(extT[k][:, c0:c1], extT_d[k * 128:(k + 1) * 128, c0:c1],
                    qs)

        # Phase A1: first Q-proj chunk's deps only (wq, bq, x cols 0-512)
        wq = load_wfused(wq_d, "wq", early)
        bq = load_bias(bq_d, "bq", early)
        load_ext(x0, x0 + 512, early)
        # Phase A2: first K-proj chunk's deps (wk, bk, ctx cols 0-512)
        wk = load_wfused(wk_d, "wk", early)
        bk = load_bias(bk_d, "bk", early)
        load_ext(0, 512, early)
        # Phase A3: V-proj weights
        wv = load_wfused(wv_d, "wv", early)
        # Phase C: ctx/pers stream (feeds kproj/vproj pacing)
        load_ext(512, x0, late)
        # Phase D: x cols for Q-proj chunks 1-3
        load_ext(x0 + 512, S, late)
        wo = []
        for m in range(2):
            t = singles.tile([128, D], bf16, tag=f"wo{m}", name=f"wo{m}")
            dma(t, wo_d[m * 128:(m + 1) * 128, :], late)
            wo.append(t)

        # persistent activation tiles
        QT = [singles.tile([128, L], bf16, tag=f"qt{m}", name=f"qt{m}")
              for m in range(2)]
        KTp = [singles.tile([128, S], bf16, tag=f"ktp{m}", name=f"ktp{m}")
               for m in range(2)]
        V = [singles.tile([128, HPC * 65], bf16, tag=f"v{st}", name=f"v{st}")
             for st in range(NST)]
        ctxT = [singles.tile([128, L], bf16, tag=f"ctx{m}", name=f"ctx{m}")
                for m in range(2)]

        # ones columns of V_aug: set once, V-proj only writes cols 0:64/head
        for st in range(NST):
            vv = V[st].rearrange("p (h c) -> p h c", c=65)
            nc.vector.memset(vv[:, :, 64:65], 1.0)
        # ones row for the tail's rank-1 reciprocal broadcast
        ones_t = singles.tile([128, 64], f32, tag="ones_t", name="ones_t")
        nc.vector.memset(ones_t[64:65, :], 1.0)

        # ---- projection helpers (chunked; psum tile passed in) ----
        def qproj_chunk(m, q, ps):
            col0 = CTX + PERS + q * 512
            for k in range(KT):
                nc.tensor.matmul(
                    out=ps[:, 0:512],
                    lhsT=wq[:, k * HDPC + m * 128:k * HDPC + (m + 1) * 128],
                    rhs=extT[k][:, col0:col0 + 512],
                    start=(k == 0), stop=(k == KT - 1),
                )
            nc.vector.tensor_scalar_add(
                out=QT[m][:, q * 512:(q + 1) * 512], in0=ps[:, 0:512],
                scalar1=bq[:, m:m + 1])

        def kproj_chunk(m, c0, cw, ps):
            for k in range(KT):
                nc.tensor.matmul(
                    out=ps[:, 0:cw],
                    lhsT=wk[:, k * HDPC + m * 128:k * HDPC + (m + 1) * 128],
                    rhs=extT[k][:, c0:c0 + cw],
                    start=(k == 0), stop=(k == KT - 1),
                )
            nc.vector.tensor_scalar_add(
                out=KTp[m][:, c0:c0 + cw], in0=ps[:, 0:cw],
                scalar1=bk[:, m:m + 1])

        def vproj_tile(st, ps):
            for k in range(KT):
                nc.tensor.matmul(
                    out=ps[:, 0:HDPC],
                    lhsT=extT[k][:, st * 128:(st + 1) * 128],
                    rhs=wv[:, k * HDPC:(k + 1) * HDPC],
                    start=(k == 0), stop=(k == KT - 1),
                )
            vview = V[st].rearrange("p (h c) -> p h c", c=65)
            nc.vector.tensor_copy(
                out=vview[:, :, 0:64],
                in_=ps[:, 0:HDPC].rearrange("p (h d) -> p h d", d=64))

        def outproj_chunk(lc, nn, ps, evac_scalar=False):
            for m in range(2):
                nc.tensor.matmul(
                    out=ps[:, 0:512],
                    lhsT=ctxT[m][:, lc * 128:(lc + 1) * 128],
                    rhs=wo[m][:, nn * 512:(nn + 1) * 512],
                    start=(m == 0), stop=(m == 1),
                )
            ot = outp.tile([128, 512], f32, tag="ot", name=f"ot{lc}_{nn}")
            if evac_scalar:
                # scalar engine is idle after the last exp: use it for
                # psum evacuation so the tail out-proj isn't DVE-bound
                nc.scalar.activation(out=ot, in_=ps[:, 0:512], func=AF.Copy)
            else:
                nc.vector.tensor_copy(out=ot, in_=ps[:, 0:512])
            oq = late[qrot[0] % len(late)]
            qrot[0] += 1
            oq.dma_start(
                out=out_d[lc * 128:(lc + 1) * 128,
                          nn * 512:(nn + 1) * 512],
                in_=ot)

        def norm_pass(mt, lq, pv_e, pv_o):
            """Evacuate pv psum (DVE), 1/sums on 128 partitions, write
            normalized bf16 ctxT quarter."""
            c0 = lq * 512
            pid = 4 * mt + lq
            pvs_e = nrm.tile([65, 512], f32, tag="pvs_e", name=f"pvse{pid}")
            pvs_o = nrm.tile([65, 512], f32, tag="pvs_o", name=f"pvso{pid}")
            rp = nrm.tile([128, 8], f32, tag="rp", name=f"rp{pid}")
            rrow = [nrm.tile([1, 512], f32, tag=f"rrow{h}",
                             name=f"rrow{h}_{pid}") for h in range(2)]
            rbro = nrm.tile([64, 1024], f32, tag="rbro", name=f"rbro{pid}")
            ctmp = nrm.tile([64, 512], bf16, tag="ctmp", name=f"ctmp{pid}")
            nc.vector.tensor_copy(out=pvs_e, in_=pv_e)
            nc.vector.tensor_copy(out=pvs_o, in_=pv_o)
            # gpsimd-triggered queue keeps this latency-critical chain out
            # of the busy sync-engine DMA stream
            nc.gpsimd.dma_start(out=sums_d[2 * pid + 0, :],
                                in_=pvs_e[64:65, :])
            nc.gpsimd.dma_start(out=sums_d[2 * pid + 1, :],
                                in_=pvs_o[64:65, :])
            for h in range(2):
                nc.gpsimd.dma_start(
                    out=rp[:, h * 4:(h + 1) * 4],
                    in_=sums_d[2 * pid + h, :].rearrange("(p o) -> p o", o=4))
            nc.vector.reciprocal(out=rp, in_=rp)
            for h in range(2):
                nc.gpsimd.dma_start(out=rrow[h],
                                    in_=rp[:, h * 4:(h + 1) * 4])
                nc.gpsimd.dma_start(
                    out=rbro[:, h * 512:(h + 1) * 512],
                    in_=rrow[h][0:1, None, :].broadcast_to([1, 64, 512]))
            nc.vector.tensor_mul(ctxT[mt][0:64, c0:c0 + 512],
                                 pvs_e[0:64, :], rbro[:, 0:512])
            nc.vector.tensor_mul(ctmp, pvs_o[0:64, :], rbro[:, 512:1024])
            nc.gpsimd.dma_start(out=ctxT[mt][64:128, c0:c0 + 512], in_=ctmp)

        def norm_tail_fused(mt, lq, pv_e, pv_o, pool):
            """Last-quarter normalization with no DMA on the critical
            path: reciprocal of the sums row via Ln/Exp on the (now idle)
            scalar engine, broadcast across partitions via a rank-1
            matmul, then the final out-proj burst."""
            c0 = lq * 512
            pvs_e = nrm.tile([65, 512], f32, tag="pvs_e", name="pvseT")
            pvs_o = nrm.tile([65, 512], f32, tag="pvs_o", name="pvsoT")
            ctmp = nrm.tile([64, 512], bf16, tag="ctmp", name="ctmpT")
            nc.vector.tensor_copy(out=pvs_e, in_=pv_e)
            nc.vector.tensor_copy(out=pvs_o, in_=pv_o)
            for pvs in (pvs_e, pvs_o):
                nc.scalar.activation(out=pvs[64:65, :], in_=pvs[64:65, :],
                                     func=AF.Ln)
                nc.scalar.activation(out=pvs[64:65, :], in_=pvs[64:65, :],
                                     func=AF.Exp, scale=-1.0)
            # quarter-2 chunks: PE work that hides the reciprocal latency
            for lc in range(8, 10):
                for nn in range(2):
                    ps = pool.tile([128, 512], f32, tag="pp", bufs=2,
                                   name=f"poq2_{lc}_{nn}")
                    outproj_chunk(lc, nn, ps)
            # broadcast 1/sums across 64 partitions: rank-1 matmul
            # ones[1,64]^T @ recip_row[1,512] (K=1, from partition 64).
            # The score ring is drained by now — borrow its psum banks.
            rbro_e = pool.tile([64, 512], f32, tag="scg", bufs=2,
                               name="rbroE", padded_shape=[128, 1024])
            rbro_o = pool.tile([64, 512], f32, tag="scg", bufs=2,
                               name="rbroO", padded_shape=[128, 1024])
            nc.tensor.matmul(out=rbro_e, lhsT=ones_t[64:65, 0:64],
                             rhs=pvs_e[64:65, :], start=True, stop=True)
            nc.tensor.matmul(out=rbro_o, lhsT=ones_t[64:65, 0:64],
                             rhs=pvs_o[64:65, :], start=True, stop=True)
            for lc in range(10, 12):
                for nn in range(2):
                    ps = pool.tile([128, 512], f32, tag="pp", bufs=2,
                                   name=f"poq2b_{lc}_{nn}")
                    outproj_chunk(lc, nn, ps)
            nc.vector.tensor_mul(ctxT[mt][0:64, c0:c0 + 512],
                                 pvs_e[0:64, :], rbro_e)
            nc.vector.tensor_mul(ctmp, pvs_o[0:64, :], rbro_o)
            nc.scalar.dma_start(out=ctxT[mt][64:128, c0:c0 + 512], in_=ctmp)
            for c in range(4):
                lc = lq * 4 + c
                for nn in range(2):
                    ps = pool.tile([128, 512], f32, tag="pp", bufs=2,
                                   name=f"pot_{lc}_{nn}")
                    outproj_chunk(lc, nn, ps)

        # ---- attention pass over one (mt, lq) quarter ----
        # Score ring: per-group double-buffered psum tiles; block i =
        # (st, parity): even-head scores then odd-head scores per s-tile.
        # QK is row-tiled: the even head's K/Q live in partitions 0-63,
        # the odd head's in 64-127, so consecutive (T0, T8) matmuls run
        # concurrently on the two 64-row PE tiles.
        # PV: emitted per completed s-tile as 4 row-tiled MMs in the fixed
        # bank-safe order (see header).
        def attn_pass(mt, lq, pname, pool, G, st_hook=None,
                      tail_pool=None, pre_emit=None):
            # pv tiles are allocated lazily at the first PV emission: the
            # previous pass's deferred norm (which reads the same psum
            # banks) must be emitted BEFORE the pool reuses them, so the
            # ring's dependency snapshot orders the new writes after it.
            pv = {}

            def get_pv():
                if not pv:
                    pv["e"] = pvp.tile([65, 512], f32, tag="pv_e",
                                       name=f"pv_e_{pname}")
                    pv["o"] = pvp.tile([65, 512], f32, tag="pv_o",
                                       name=f"pv_o_{pname}")
                return pv["e"], pv["o"]
            q0 = lq * 512
            blocks = [(st, par) for st in range(NST) for par in (0, 1)]
            groups = [list(enumerate(blocks))[g:g + G]
                      for g in range(0, len(blocks), G)]
            seen_st = set()
            pend_sts = []
            sctiles = {}

            def emit_qk_group(gj):
                grp = groups[gj]
                # separate double-buffered tile per group: the WAR for
                # group j's writes lands on exp(j-2) (2-back reader), not
                # on the previous exp (slicing one big ring tensor gave
                # whole-tensor WAR deps and serialized the pipeline)
                sg = pool.tile([128, len(grp) * 512], f32, tag="scg",
                               bufs=2, name=f"sc_{pname}_{gj}",
                               padded_shape=[128, G * 512])
                sctiles[gj] = sg
                for jj, (i, (st, par)) in enumerate(grp):
                    if par == 0 and st not in seen_st:
                        seen_st.add(st)
                        pend_sts.append(st)
                    p0 = par * 64
                    nc.tensor.matmul(
                        out=sg[:, jj * 512:(jj + 1) * 512],
                        lhsT=KTp[mt][p0:p0 + 64, st * 128:(st + 1) * 128],
                        rhs=QT[mt][p0:p0 + 64, q0:q0 + 512],
                        start=True, stop=True,
                    )

            def run_hooks():
                # hook work (projections: untiled matmuls) batched next to
                # the PV batch so QK<->PV/proj PE tiling-mode switches
                # happen once per batch, not per chunk
                while pend_sts:
                    st = pend_sts.pop(0)
                    if st_hook is not None:
                        st_hook(st)

            # Software pipeline: per j emit exp(j), QK(j+1); every second
            # j also hooks + a PV batch covering the last two exp'd
            # groups. PV lags exp enough that the in-order PE queue never
            # stalls on an activation, and QK (row-tiled) runs in longer
            # uninterrupted stretches between untiled PV/proj batches.
            eslice = {}      # block index -> (e tile, col0)
            pv_next = [0]    # next s-tile to emit PV for

            def emit_pv_ready(limit_blk):
                while pv_next[0] < NST:
                    st = pv_next[0]
                    b0, b1 = 2 * st, 2 * st + 1
                    if b1 > limit_blk or b0 not in eslice:
                        return
                    pv_e, pv_o = get_pv()
                    e0, c0_ = eslice.pop(b0)
                    e1, c1_ = eslice.pop(b1)
                    h0 = 2 * mt
                    h1 = 2 * mt + 1
                    first = (st == 0)
                    last = (st == NST - 1)
                    nc.tensor.matmul(
                        out=pv_e, lhsT=V[st][:, h0 * 65:h0 * 65 + 65],
                        rhs=e0[:, c0_:c0_ + 512],
                        start=first, stop=last)
                    nc.tensor.matmul(
                        out=pv_o, lhsT=V[st][:, h1 * 65:h1 * 65 + 65],
                        rhs=e1[:, c1_:c1_ + 512],
                        start=first, stop=last)
                    pv_next[0] += 1

            emit_qk_group(0)
            for j, grp in enumerate(groups):
                gw = len(grp) * 512
                e = esb.tile([128, gw], bf16, tag=f"e{G}",
                             name=f"e_{pname}_{j}")
                nc.scalar.activation(
                    out=e, in_=sctiles.pop(j)[:, 0:gw],
                    func=AF.Exp, scale=float(SCALE))
                for jj, (bi, _) in enumerate(grp):
                    eslice[bi] = (e, jj * 512)
                if j == 0 and pre_emit is not None:
                    pre_emit()
                if j + 1 < len(groups):
                    emit_qk_group(j + 1)
                if j % 2 == 1 and j - 1 >= 0:
                    run_hooks()
                    emit_pv_ready(groups[j - 1][-1][0])
            run_hooks()
            emit_pv_ready(2 * NST)
            pv_e, pv_o = get_pv()
            if tail_pool is not None:
                norm_tail_fused(mt, lq, pv_e, pv_o, tail_pool)
                return None
            return (mt, lq, pv_e, pv_o)

        # ---- ancillary work queues ----
        kdone = [0]

        def kproj_m0_upto(scol, pool):
            while kdone[0] < min(scol, S):
                cw = min(512, S - kdone[0])
                ps = pool.tile([128, 512], f32, tag="pp", bufs=2,
                               name=f"psk0_{kdone[0]}")
                kproj_chunk(0, kdone[0], cw, ps)
                kdone[0] += cw

        # ======== passes ========
        # Norm of pass i is deferred into the start of pass i+1, so pool
        # transitions never wait the norm's DMA-latency chain and the next
        # pass's attention starts immediately.
        pend = {"n": None}

        def flush_norm():
            if pend["n"] is not None:
                norm_pass(*pend["n"])
                pend["n"] = None

        def run_pass(mt, lq, pname, pool, G, st_hook=None, tail_pool=None):
            r = attn_pass(mt, lq, pname, pool, G, st_hook, tail_pool,
                          pre_emit=flush_norm)
            if r is not None:
                pend["n"] = r

        # anc passes (G=2, exp N=1024, pp proj banks alive):
        #   (0,0): vproj + kproj-m0 pacing + qproj(0,1)
        #   (0,1): kproj-m1 chunks
        #   (0,2): remaining qproj chunks
        # pure passes (G=3, exp N=1536): (0,3) (1,0) (1,1) (1,2)
        # last pass (1,3): anc, out-proj hooks + fused norm tail
        with tc.tile_pool(name="pha", bufs=1, space="PSUM") as pha:
            # warm-up matmuls on already-memset SBUF while the input DMA
            # stream lands: ~4us of PE activity flips the HAM clock gate
            # to 2.4 GHz before the first real projections run
            wps = pha.tile([128, 256], f32, tag="pp", bufs=2, name="warmps")
            for w in range(24):
                nc.tensor.matmul(
                    out=wps, lhsT=V[0][:, 0:128], rhs=V[1][:, 0:256],
                    start=True, stop=True)
            ps = pha.tile([128, 512], f32, tag="pp", bufs=2, name="psq00")
            qproj_chunk(0, 0, ps)
            kproj_m0_upto(512, pha)

            def hook00(st):
                kproj_m0_upto((st + 4) * 128, pha)
                if st == 0:
                    for pre in range(2):
                        psv = pha.tile([128, 512], f32, tag="pp", bufs=2,
                                       name=f"psv{pre}")
                        vproj_tile(pre, psv)
                if st + 2 < NST:
                    psv = pha.tile([128, 512], f32, tag="pp", bufs=2,
                                   name=f"psv{st + 2}")
                    vproj_tile(st + 2, psv)
                if st == 30:
                    psq = pha.tile([128, 512], f32, tag="pp", bufs=2,
                                   name="psq01")
                    qproj_chunk(0, 1, psq)

            run_pass(0, 0, "p00", pha, 2, hook00)

            # (0,1) hooks: its successor's Q chunk first (a pass's own QT
            # quarter must be projected in an EARLIER pass — its first QK
            # groups are emitted before any of its own hooks run), then
            # the m1 K projection.
            ex01 = [("q", 0, 2)] + [
                ("k1", c * 512, min(512, S - c * 512))
                for c in range((S + 511) // 512)]

            def hook01(st):
                if ex01 and st % 3 == 0:
                    kind, a, b = ex01.pop(0)
                    ps = pha.tile([128, 512], f32, tag="pp", bufs=2,
                                  name=f"exA_{kind}_{a}_{b}")
                    if kind == "q":
                        qproj_chunk(a, b, ps)
                    else:
                        kproj_chunk(1, a, b, ps)

            run_pass(0, 1, "p01", pha, 2, hook01)
            while ex01:
                kind, a, b = ex01.pop(0)
                ps = pha.tile([128, 512], f32, tag="pp", bufs=2,
                              name=f"exAL_{a}_{b}")
                if kind == "q":
                    qproj_chunk(a, b, ps)
                else:
                    kproj_chunk(1, a, b, ps)

            ex02 = [(0, 3), (1, 0), (1, 1), (1, 2), (1, 3)]

            def hook02(st):
                if ex02 and st % 4 == 0:
                    m, qq = ex02.pop(0)
                    ps = pha.tile([128, 512], f32, tag="pp", bufs=2,
                                  name=f"exB_q{m}_{qq}")
                    qproj_chunk(m, qq, ps)

            run_pass(0, 2, "p02", pha, 2, hook02)
            assert not ex02, f"{len(ex02)} ancillary items left"

        # pure passes: G=3, exp N=1536
        with tc.tile_pool(name="phf", bufs=1, space="PSUM") as phf:
            for mt, lq, pname in [(0, 3, "p03"), (1, 0, "p10"),
                                  (1, 1, "p11"), (1, 2, "p12")]:
                run_pass(mt, lq, pname, phf, 3)

        # last pass: anc type; interleave out-proj for L quarters 0-1
        # (both pairs done for those), fused tail covers 2-3.
        with tc.tile_pool(name="phl", bufs=1, space="PSUM") as phl:
            outq = [(lc, nn) for lc in range(8) for nn in range(2)]

            def hook13(st):
                if outq:
                    lc, nn = outq.pop(0)
                    ps = phl.tile([128, 512], f32, tag="pp", bufs=2,
                                  name=f"po_{lc}_{nn}")
                    outproj_chunk(lc, nn, ps)

            run_pass(1, 3, "p13", phl, 2, hook13, tail_pool=phl)

    nself = (0 if os.environ.get("KSELFWAIT") == "keep"
             else _drop_self_waits(nc))
    ndrop = _dedupe_ldweights(nc)
    nsplit = _split_multiwaits(nc)
    if os.environ.get("KVERBOSE"):
        print(f"[kernel] dropped {ndrop} redundant ldweights, "
              f"{nself} self-waits, split {nsplit} multi-wait instrs")
    _BUILT[0] = nc
    return nc


def kernel(**inputs):
    global LAST_EXEC_TIME_NS
    from concourse import bass_utils

    x = np.asarray(inputs["x"], np.float32)
    ctx_mem = np.asarray(inputs["ctx_mem"], np.float32)
    pers_mem = np.asarray(inputs["pers_mem"], np.float32)
    Wq = np.asarray(inputs["Wq"], np.float32)
    Wk = np.asarray(inputs["Wk"], np.float32)
    Wv = np.asarray(inputs["Wv"], np.float32)
    Wo = np.asarray(inputs["Wo"], np.float32)
    bq = np.asarray(inputs["bq"], np.float32)
    bk = np.asarray(inputs["bk"], np.float32)
    bv = np.asarray(inputs["bv"], np.float32)
    bo = np.asarray(inputs["bo"], np.float32)

    nc = _build()

    extT_b = []
    for b in range(B):
        ext = np.concatenate([ctx_mem, pers_mem, x[b]], axis=0)  # [S, D]
        extT_b.append(np.ascontiguousarray(ext.T).astype(BF16))

    wq_bf = Wq.astype(BF16)
    wk_bf = Wk.astype(BF16)
    wv_bf = Wv.astype(BF16)
    wo_bf = Wo.astype(BF16)

    in_maps = []
    for c in range(NCORES):
        b, g = divmod(c, NCORES // B)
        cols = slice(g * HDPC, (g + 1) * HDPC)
        in_maps.append({
            "extT": extT_b[b],
            "wq": np.ascontiguousarray(wq_bf[:, cols]),
            "wk": np.ascontiguousarray(wk_bf[:, cols]),
            "wv": np.ascontiguousarray(wv_bf[:, cols]),
            "wo": np.ascontiguousarray(wo_bf[cols, :]),
            "bq": np.ascontiguousarray(bq[cols]),
            "bk": np.ascontiguousarray(bk[cols]),
        })

    res = bass_utils.run_bass_kernel_spmd(
        nc, in_maps, core_ids=list(range(NCORES)),
        trace=bool(os.environ.get("KPROF")),
    )
    LAST_EXEC_TIME_NS = res.exec_time_ns

    if os.environ.get("KDEBUG_CORES"):
        for c in range(NCORES):
            o = res.results[c]["out"]
            nanmask = ~np.isfinite(o)
            if nanmask.any():
                rows = np.where(nanmask.any(axis=1))[0]
                cols = np.where(nanmask.any(axis=0))[0]
                print(f"[core {c}] bad frac={nanmask.mean():.4f} "
                      f"rows [{rows.min()}..{rows.max()}] n={len(rows)} "
                      f"cols [{cols.min()}..{cols.max()}] n={len(cols)}")

    out = np.zeros((B, L, D), np.float32)
    for c in range(NCORES):
        b = c // (NCORES // B)
        out[b] += res.results[c]["out"]
    out += (bo + bv.astype(np.float32) @ Wo)[None, None, :]
    return out


# revision 39
# speedup vs baseline: 1.0001x; 1.0001x over previous
# Trainium2 Bass kernel for MemoryAttention (B=2, L=2048, D=1024, H=16, HD=64,
# CTX=2048, PERS=256 -> S=4352), sharded over 8 NeuronCores as
# (batch, head-group-of-4). Self-contained: hardcodes all shapes.
#
# v7: row-tiled PE (64x128 tiling) for QK: each 64-row tile of the PE
# array computes one head's contraction (head-dim=64) concurrently with
# the other head on the other tile, so a QK pair (both heads, one s-tile)
# streams in ~512 cycles instead of 2x512. PV stays untiled (K=128 over
# s; cross-tile accumulation into one PSUM bank hangs the device), and
# is batched every second exp group together with the projection hooks
# so PE tiling-mode switches amortize. Norm of each quarter is deferred
# into the next pass (off the pool-transition barrier); the last-quarter
# norm uses Ln/Exp reciprocal + a rank-1 matmul broadcast (no DMA
# round-trip on the tail critical path). Input DMAs are staged in
# dependency order (Q-proj prefix, K-proj prefix, V weights, streamed
# ctx/x) across all three trigger queues; warm-up matmuls keep the PE
# HAM clock-gate at 2.4 GHz through the DMA-bound startup.
#
# Per-core layout:
#   extT  [D, S]    bf16  (ext = [ctx; pers; x_b], transposed on host)
#   QT_m  [128, L]  = (x Wq + bq)^T   m-tile rows: head 2m dims 0-63,
#                     head 2m+1 dims 64-127
#   KT_m  [128, S]  = (ext Wk + bk)^T same row layout as QT (head pair)
#   V     [S, 4*65] = ext Wv  (+ ones column per head for softmax sums)
#   sc    PSUM ring: per s-tile, even-head scores then odd-head scores
#   e     = exp(0.125 * sc)  (no max-subtraction: |scores|<=~9 here, exp is
#                     exact-safe in f32)
#   pv_h  [65, 1024] = sum_s V_aug[s]^T E[s]: rows 0..63 ctx^T, row 64 sums
#   ctxT_m[128, L]  = pv pair * (1/sums)  (bf16)
#   out   [L, D]    = sum_m ctxT_m^T Wo_m (+ host adds bo + bv@Wo)
import os
import numpy as np
import ml_dtypes

D = 1024
HD = 64
B = 2
L = 2048
CTX = 2048
PERS = 256
S = CTX + PERS + L          # 4352
NCORES = 8
HPC = 4                     # heads per core
HDPC = HPC * HD             # 256
KT = D // 128               # 8 k-tiles
NST = S // 128              # 34 s-tiles
SCALE = 1.0 / np.sqrt(HD)   # 0.125

BF16 = ml_dtypes.bfloat16

_BUILT = {}
LAST_EXEC_TIME_NS = None


def _split_multiwaits(nc):
    """This walrus build accepts at most ONE sync-wait command per engine
    instruction (2 for EventSemaphore). Tile emits instructions with several
    waits (and a closing drain with one wait per live proc). Legalize by
    hoisting extra waits onto same-engine NoOps inserted just before the
    instruction — strictly more conservative ordering, so still correct."""
    import concourse.mybir as mybir

    ctr = [0]
    for fn in nc.m.functions:
        for bb in fn.blocks:
            changed = False
            new = []
            for inst in bb.instructions:
                si = inst.sync_info
                limit = 2 if isinstance(inst, mybir.InstEventSemaphore) else 1
                if si is not None and si.on_wait and len(si.on_wait) > limit:
                    waits = list(si.on_wait)
                    for w in waits[:-limit]:
                        ctr[0] += 1
                        nop = mybir.InstNoOp(
                            name=f"wsplit_{ctr[0]}",
                            engine=inst.engine,
                            sync_info=mybir.SyncInfo(
                                on_wait=[w], on_update=[]),
                        )
                        new.append(nop)
                    si.on_wait = waits[-limit:]
                    changed = True
                new.append(inst)
            if changed:
                bb.instructions = new
    return ctr[0]


def _drop_self_waits(nc):
    """Drop sem waits where a compute-engine instruction waits on its OWN
    engine's completion sem (WAW/WAR vs an earlier same-engine op): engines
    execute their queue in order, one op at a time, so these are satisfied
    by construction. Not applied to SP/DMA (async HWDGE queues) or
    barrier/drain instructions."""
    import concourse.mybir as mybir

    eng_sem = {
        "EngineType.PE": "PE_",
        "EngineType.Activation": "Activation_",
        "EngineType.DVE": "DVE_",
        "EngineType.Pool": "Pool_",
    }
    dropped = 0
    for fn in nc.m.functions:
        for bb in fn.blocks:
            for inst in bb.instructions:
                nm = type(inst).__name__
                if nm in ("InstDrain", "InstEventSemaphore", "InstNoOp",
                          "InstDMACopy"):
                    continue
                pref = eng_sem.get(str(inst.engine))
                si = inst.sync_info
                if pref is None or si is None or not si.on_wait:
                    continue
                kept = [w for w in si.on_wait
                        if not (w.ant_name or "").startswith(pref)]
                if len(kept) != len(si.on_wait):
                    dropped += len(si.on_wait) - len(kept)
                    si.on_wait = kept
    return dropped


def _dedupe_ldweights(nc):
    """Tile lowering emits one InstLdweights per InstMatmult even when
    consecutive matmuls use the identical stationary operand. Drop the
    redundant reloads (same weights AP + tile_position, no sem updates,
    nothing but matmuls/ldweights in between on PE)."""
    import concourse.mybir as mybir

    def ldw_key(inst):
        try:
            ap = inst.ins[0].bass_ap
            return (str(ap.tensor.name), ap.offset, str(ap.ap),
                    str(getattr(inst, "tile_position", None)),
                    str(getattr(inst, "perf_mode", None)),
                    str(getattr(inst, "is_transpose", None)))
        except Exception:
            return None

    dropped = 0
    for fn in nc.m.functions:
        for bb in fn.blocks:
            last_key = None
            new = []
            for inst in bb.instructions:
                if str(inst.engine) != "EngineType.PE":
                    new.append(inst)
                    continue
                nm = type(inst).__name__
                if nm == "InstLdweights":
                    si = inst.sync_info
                    has_upd = bool(si and si.on_update)
                    k = ldw_key(inst)
                    if k is not None and k == last_key and not has_upd:
                        if si and si.on_wait:
                            nop = mybir.InstNoOp(
                                name=f"{inst.name}_ldwkeep",
                                engine=inst.engine,
                                sync_info=mybir.SyncInfo(
                                    on_wait=list(si.on_wait), on_update=[]),
                            )
                            new.append(nop)
                        dropped += 1
                        continue
                    last_key = k
                    new.append(inst)
                elif nm == "InstMatmult":
                    new.append(inst)
                else:
                    last_key = None
                    new.append(inst)
            bb.instructions = new
    return dropped


def _build():
    if 0 in _BUILT:
        return _BUILT[0]

    from contextlib import ExitStack as _ES

    import concourse.bass as bass
    import concourse.mybir as mybir
    import concourse.tile as tile

    f32 = mybir.dt.float32
    bf16 = mybir.dt.bfloat16
    AF = mybir.ActivationFunctionType

    nc = bass.Bass(trn_type="TRN2")
    extT_d = nc.dram_tensor("extT", [D, S], bf16, kind="ExternalInput")
    wq_d = nc.dram_tensor("wq", [D, HDPC], bf16, kind="ExternalInput")
    wk_d = nc.dram_tensor("wk", [D, HDPC], bf16, kind="ExternalInput")
    wv_d = nc.dram_tensor("wv", [D, HDPC], bf16, kind="ExternalInput")
    wo_d = nc.dram_tensor("wo", [HDPC, D], bf16, kind="ExternalInput")
    bq_d = nc.dram_tensor("bq", [HDPC], f32, kind="ExternalInput")
    bk_d = nc.dram_tensor("bk", [HDPC], f32, kind="ExternalInput")
    out_d = nc.dram_tensor("out", [L, D], f32, kind="ExternalOutput")
    # scratch for bouncing [1,512] softmax sums into a [128,4] layout
    sums_d = nc.dram_tensor("sums_scratch", [16, 512], f32, kind="Internal")

    with tile.TileContext(nc) as tc, _ES() as es:
        singles = es.enter_context(tc.tile_pool(name="singles", bufs=1))
        # pv pool (2 banks) lives for the whole kernel; pass-scoped pools
        # take the other 6.
        pvp = es.enter_context(tc.tile_pool(name="pvp", bufs=1, space="PSUM"))
        esb = es.enter_context(tc.tile_pool(name="esb", bufs=4))
        nrm = es.enter_context(tc.tile_pool(name="nrm", bufs=3))
        outp = es.enter_context(tc.tile_pool(name="outsb", bufs=4))

        # -------- inputs: staged DMA with minimal gating prefix --------
        # Phase A: what the very first Q-proj + K-proj + first QK need.
        # Phase B: ctx cols for K/V-proj start.  Phase C: ctx/pers stream
        # for the kproj/vproj pacing.  Phase D: x cols for later Q-proj
        # chunks.  Queues round-robin sync/gpsimd/vector (+scalar early,
        # before the first activation needs the scalar queue).
        x0 = CTX + PERS
        qrot = [0]
        early = [nc.sync, nc.scalar, nc.gpsimd]
        late = [nc.sync, nc.gpsimd]

        def dma(out, in_, qs):
            q = qs[qrot[0] % len(qs)]
            qrot[0] += 1
            q.dma_start(out=out, in_=in_)

        def load_wfused(dram, name, qs):
            # all 8 k-tiles of a [D, HDPC] weight side by side in one
            # [128, 8*256] tile; two 3d-AP DMAs (4 k-tiles each) so the
            # byte load balances across the trigger queues
            t = singles.tile([128, KT * HDPC], bf16, tag=name, name=name)
            tv = t.rearrange("p (k c) -> p k c", k=KT)
            dv = dram.rearrange("(k p) c -> p k c", p=128)
            dma(tv[:, 0:4, :], dv[:, 0:4, :], qs)
            dma(tv[:, 4:8, :], dv[:, 4:8, :], qs)
            return t

        def load_bias(dram, name, qs):
            t = singles.tile([128, 2], f32, tag=name, name=name)
            for m in range(2):
                dma(t[:, m:m + 1],
                    dram[m * 128:(m + 1) * 128].rearrange(
                        "(p o) -> p o", o=1), qs)
            return t

        extT = [singles.tile([128, S], bf16, tag=f"extT{k}",
                             name=f"extT{k}") for k in range(KT)]

        def load_ext(c0, c1, qs):
            for k in range(KT):
                dma(extT[k][:, c0:c1], extT_d[k * 128:(k + 1) * 128, c0:c1],
                    qs)

        # Phase A1: first Q-proj chunk's deps only (wq, bq, x cols 0-512)
        wq = load_wfused(wq_d, "wq", early)
        bq = load_bias(bq_d, "bq", early)
        load_ext(x0, x0 + 512, early)
        # Phase A2: first K-proj chunk's deps (wk, bk, ctx cols 0-512)
        wk = load_wfused(wk_d, "wk", early)
        bk = load_bias(bk_d, "bk", early)
        load_ext(0, 512, early)
        # Phase A3: V-proj weights
        wv = load_wfused(wv_d, "wv", early)
        # Phase C: ctx/pers stream (feeds kproj/vproj pacing)
        load_ext(512, x0, late)
        # Phase D: x cols for Q-proj chunks 1-3
        load_ext(x0 + 512, S, late)
        wo = []
        for m in range(2):
            t = singles.tile([128, D], bf16, tag=f"wo{m}", name=f"wo{m}")
            dma(t, wo_d[m * 128:(m + 1) * 128, :], late)
            wo.append(t)

        # persistent activation tiles
        QT = [singles.tile([128, L], bf16, tag=f"qt{m}", name=f"qt{m}")
              for m in range(2)]
        KTp = [singles.tile([128, S], bf16, tag=f"ktp{m}", name=f"ktp{m}")
               for m in range(2)]
        V = [singles.tile([128, HPC * 65], bf16, tag=f"v{st}", name=f"v{st}")
             for st in range(NST)]
        ctxT = [singles.tile([128, L], bf16, tag=f"ctx{m}", name=f"ctx{m}")
                for m in range(2)]

        # ones columns of V_aug: set once, V-proj only writes cols 0:64/head
        for st in range(NST):
            vv = V[st].rearrange("p (h c) -> p h c", c=65)
            nc.vector.memset(vv[:, :, 64:65], 1.0)
        # ones row for the tail's rank-1 reciprocal broadcast
        ones_t = singles.tile([128, 64], f32, tag="ones_t", name="ones_t")
        nc.vector.memset(ones_t[64:65, :], 1.0)

        # ---- projection helpers (chunked; psum tile passed in) ----
        def qproj_chunk(m, q, ps):
            col0 = CTX + PERS + q * 512
            for k in range(KT):
                nc.tensor.matmul(
                    out=ps[:, 0:512],
                    lhsT=wq[:, k * HDPC + m * 128:k * HDPC + (m + 1) * 128],
                    rhs=extT[k][:, col0:col0 + 512],
                    start=(k == 0), stop=(k == KT - 1),
                )
            nc.vector.tensor_scalar_add(
                out=QT[m][:, q * 512:(q + 1) * 512], in0=ps[:, 0:512],
                scalar1=bq[:, m:m + 1])

        def kproj_chunk(m, c0, cw, ps):
            for k in range(KT):
                nc.tensor.matmul(
                    out=ps[:, 0:cw],
                    lhsT=wk[:, k * HDPC + m * 128:k * HDPC + (m + 1) * 128],
                    rhs=extT[k][:, c0:c0 + cw],
                    start=(k == 0), stop=(k == KT - 1),
                )
            nc.vector.tensor_scalar_add(
                out=KTp[m][:, c0:c0 + cw], in0=ps[:, 0:cw],
                scalar1=bk[:, m:m + 1])

        def vproj_tile(st, ps):
            for k in range(KT):
                nc.tensor.matmul(
                    out=ps[:, 0:HDPC],
                    lhsT=extT[k][:, st * 128:(st + 1) * 128],
                    rhs=wv[:, k * HDPC:(k + 1) * HDPC],
                    start=(k == 0), stop=(k == KT - 1),
                )
            vview = V[st].rearrange("p (h c) -> p h c", c=65)
            nc.vector.tensor_copy(
                out=vview[:, :, 0:64],
                in_=ps[:, 0:HDPC].rearrange("p (h d) -> p h d", d=64))

        def outproj_chunk(lc, nn, ps, evac_scalar=False):
            for m in range(2):
                nc.tensor.matmul(
                    out=ps[:, 0:512],
                    lhsT=ctxT[m][:, lc * 128:(lc + 1) * 128],
                    rhs=wo[m][:, nn * 512:(nn + 1) * 512],
                    start=(m == 0), stop=(m == 1),
                )
            ot = outp.tile([128, 512], f32, tag="ot", name=f"ot{lc}_{nn}")
            if evac_scalar:
                # scalar engine is idle after the last exp: use it for
                # psum evacuation so the tail out-proj isn't DVE-bound
                nc.scalar.activation(out=ot, in_=ps[:, 0:512], func=AF.Copy)
            else:
                nc.vector.tensor_copy(out=ot, in_=ps[:, 0:512])
            oq = late[qrot[0] % len(late)]
            qrot[0] += 1
            oq.dma_start(
                out=out_d[lc * 128:(lc + 1) * 128,
                          nn * 512:(nn + 1) * 512],
                in_=ot)

        def norm_pass(mt, lq, pv_e, pv_o):
            """Evacuate pv psum (DVE), 1/sums on 128 partitions, write
            normalized bf16 ctxT quarter."""
            c0 = lq * 512
            pid = 4 * mt + lq
            pvs_e = nrm.tile([65, 512], f32, tag="pvs_e", name=f"pvse{pid}")
            pvs_o = nrm.tile([65, 512], f32, tag="pvs_o", name=f"pvso{pid}")
            rp = nrm.tile([128, 8], f32, tag="rp", name=f"rp{pid}")
            rrow = [nrm.tile([1, 512], f32, tag=f"rrow{h}",
                             name=f"rrow{h}_{pid}") for h in range(2)]
            rbro = nrm.tile([64, 1024], f32, tag="rbro", name=f"rbro{pid}")
            ctmp = nrm.tile([64, 512], bf16, tag="ctmp", name=f"ctmp{pid}")
            nc.vector.tensor_copy(out=pvs_e, in_=pv_e)
            nc.vector.tensor_copy(out=pvs_o, in_=pv_o)
            # gpsimd-triggered queue keeps this latency-critical chain out
            # of the busy sync-engine DMA stream
            nc.gpsimd.dma_start(out=sums_d[2 * pid + 0, :],
                                in_=pvs_e[64:65, :])
            nc.gpsimd.dma_start(out=sums_d[2 * pid + 1, :],
                                in_=pvs_o[64:65, :])
            for h in range(2):
                nc.gpsimd.dma_start(
                    out=rp[:, h * 4:(h + 1) * 4],
                    in_=sums_d[2 * pid + h, :].rearrange("(p o) -> p o", o=4))
            nc.vector.reciprocal(out=rp, in_=rp)
            for h in range(2):
                nc.gpsimd.dma_start(out=rrow[h],
                                    in_=rp[:, h * 4:(h + 1) * 4])
                nc.gpsimd.dma_start(
                    out=rbro[:, h * 512:(h + 1) * 512],
                    in_=rrow[h][0:1, None, :].broadcast_to([1, 64, 512]))
            nc.vector.tensor_mul(ctxT[mt][0:64, c0:c0 + 512],
                                 pvs_e[0:64, :], rbro[:, 0:512])
            nc.vector.tensor_mul(ctmp, pvs_o[0:64, :], rbro[:, 512:1024])
            nc.gpsimd.dma_start(out=ctxT[mt][64:128, c0:c0 + 512], in_=ctmp)

        def norm_tail_fused(mt, lq, pv_e, pv_o, pool):
            """Last-quarter normalization with no DMA on the critical
            path: reciprocal of the sums row via Ln/Exp on the (now idle)
            scalar engine, broadcast across partitions via a rank-1
            matmul, then the final out-proj burst."""
            c0 = lq * 512
            pvs_e = nrm.tile([65, 512], f32, tag="pvs_e", name="pvseT")
            pvs_o = nrm.tile([65, 512], f32, tag="pvs_o", name="pvsoT")
            ctmp = nrm.tile([64, 512], bf16, tag="ctmp", name="ctmpT")
            nc.vector.tensor_copy(out=pvs_e, in_=pv_e)
            nc.vector.tensor_copy(out=pvs_o, in_=pv_o)
            for pvs in (pvs_e, pvs_o):
                nc.scalar.activation(out=pvs[64:65, :], in_=pvs[64:65, :],
                                     func=AF.Ln)
                nc.scalar.activation(out=pvs[64:65, :], in_=pvs[64:65, :],
                                     func=AF.Exp, scale=-1.0)
            # quarter-2 chunks: PE work that hides the reciprocal latency
            for lc in range(8, 10):
                for nn in range(2):
                    ps = pool.tile([128, 512], f32, tag="pp", bufs=2,
                                   name=f"poq2_{lc}_{nn}")
                    outproj_chunk(lc, nn, ps)
            # broadcast 1/sums across 64 partitions: rank-1 matmul
            # ones[1,64]^T @ recip_row[1,512] (K=1, from partition 64).
            # The score ring is drained by now — borrow its psum banks.
            rbro_e = pool.tile([64, 512], f32, tag="scg", bufs=2,
                               name="rbroE", padded_shape=[128, 1024])
            rbro_o = pool.tile([64, 512], f32, tag="scg", bufs=2,
                               name="rbroO", padded_shape=[128, 1024])
            nc.tensor.matmul(out=rbro_e, lhsT=ones_t[64:65, 0:64],
                             rhs=pvs_e[64:65, :], start=True, stop=True)
            nc.tensor.matmul(out=rbro_o, lhsT=ones_t[64:65, 0:64],
                             rhs=pvs_o[64:65, :], start=True, stop=True)
            for lc in range(10, 12):
                for nn in range(2):
                    ps = pool.tile([128, 512], f32, tag="pp", bufs=2,
                                   name=f"poq2b_{lc}_{nn}")
                    outproj_chunk(lc, nn, ps)
            nc.vector.tensor_mul(ctxT[mt][0:64, c0:c0 + 512],
                                 pvs_e[0:64, :], rbro_e)
            nc.vector.tensor_mul(ctmp, pvs_o[0:64, :], rbro_o)
            nc.scalar.dma_start(out=ctxT[mt][64:128, c0:c0 + 512], in_=ctmp)
            for c in range(4):
                lc = lq * 4 + c
                for nn in range(2):
                    ps = pool.tile([128, 512], f32, tag="pp", bufs=2,
                                   name=f"pot_{lc}_{nn}")
                    outproj_chunk(lc, nn, ps)

        # ---- attention pass over one (mt, lq) quarter ----
        # Score ring: per-group double-buffered psum tiles; block i =
        # (st, parity): even-head scores then odd-head scores per s-tile.
        # QK is row-tiled: the even head's K/Q live in partitions 0-63,
        # the odd head's in 64-127, so consecutive (T0, T8) matmuls run
        # concurrently on the two 64-row PE tiles.
        # PV: emitted per completed s-tile as 4 row-tiled MMs in the fixed
        # bank-safe order (see header).
        def attn_pass(mt, lq, pname, pool, G, st_hook=None,
                      tail_pool=None, pre_emit=None):
            # pv tiles are allocated lazily at the first PV emission: the
            # previous pass's deferred norm (which reads the same psum
            # banks) must be emitted BEFORE the pool reuses them, so the
            # ring's dependency snapshot orders the new writes after it.
            pv = {}

            def get_pv():
                if not pv:
                    pv["e"] = pvp.tile([65, 512], f32, tag="pv_e",
                                       name=f"pv_e_{pname}")
                    pv["o"] = pvp.tile([65, 512], f32, tag="pv_o",
                                       name=f"pv_o_{pname}")
                return pv["e"], pv["o"]
            q0 = lq * 512
            blocks = [(st, par) for st in range(NST) for par in (0, 1)]
            groups = [list(enumerate(blocks))[g:g + G]
                      for g in range(0, len(blocks), G)]
            seen_st = set()
            pend_sts = []
            sctiles = {}

            def emit_qk_group(gj):
                grp = groups[gj]
                # separate double-buffered tile per group: the WAR for
                # group j's writes lands on exp(j-2) (2-back reader), not
                # on the previous exp (slicing one big ring tensor gave
                # whole-tensor WAR deps and serialized the pipeline)
                sg = pool.tile([128, len(grp) * 512], f32, tag="scg",
                               bufs=2, name=f"sc_{pname}_{gj}",
                               padded_shape=[128, G * 512])
                sctiles[gj] = sg
                for jj, (i, (st, par)) in enumerate(grp):
                    if par == 0 and st not in seen_st:
                        seen_st.add(st)
                        pend_sts.append(st)
                    p0 = par * 64
                    nc.tensor.matmul(
                        out=sg[:, jj * 512:(jj + 1) * 512],
                        lhsT=KTp[mt][p0:p0 + 64, st * 128:(st + 1) * 128],
                        rhs=QT[mt][p0:p0 + 64, q0:q0 + 512],
                        start=True, stop=True,
                    )

            def run_hooks():
                # hook work (projections: untiled matmuls) batched next to
                # the PV batch so QK<->PV/proj PE tiling-mode switches
                # happen once per batch, not per chunk
                while pend_sts:
                    st = pend_sts.pop(0)
                    if st_hook is not None:
                        st_hook(st)

            # Software pipeline: per j emit exp(j), QK(j+1); every second
            # j also hooks + a PV batch covering the last two exp'd
            # groups. PV lags exp enough that the in-order PE queue never
            # stalls on an activation, and QK (row-tiled) runs in longer
            # uninterrupted stretches between untiled PV/proj batches.
            eslice = {}      # block index -> (e tile, col0)
            pv_next = [0]    # next s-tile to emit PV for

            def emit_pv_ready(limit_blk):
                while pv_next[0] < NST:
                    st = pv_next[0]
                    b0, b1 = 2 * st, 2 * st + 1
                    if b1 > limit_blk or b0 not in eslice:
                        return
                    pv_e, pv_o = get_pv()
                    e0, c0_ = eslice.pop(b0)
                    e1, c1_ = eslice.pop(b1)
                    h0 = 2 * mt
                    h1 = 2 * mt + 1
                    first = (st == 0)
                    last = (st == NST - 1)
                    nc.tensor.matmul(
                        out=pv_e, lhsT=V[st][:, h0 * 65:h0 * 65 + 65],
                        rhs=e0[:, c0_:c0_ + 512],
                        start=first, stop=last)
                    nc.tensor.matmul(
                        out=pv_o, lhsT=V[st][:, h1 * 65:h1 * 65 + 65],
                        rhs=e1[:, c1_:c1_ + 512],
                        start=first, stop=last)
                    pv_next[0] += 1

            emit_qk_group(0)
            for j, grp in enumerate(groups):
                gw = len(grp) * 512
                e = esb.tile([128, gw], bf16, tag=f"e{G}",
                             name=f"e_{pname}_{j}")
                nc.scalar.activation(
                    out=e, in_=sctiles.pop(j)[:, 0:gw],
                    func=AF.Exp, scale=float(SCALE))
                for jj, (bi, _) in enumerate(grp):
                    eslice[bi] = (e, jj * 512)
                if j == 0 and pre_emit is not None:
                    pre_emit()
                if j + 1 < len(groups):
                    emit_qk_group(j + 1)
                if j % 2 == 1 and j - 1 >= 0:
                    run_hooks()
                    emit_pv_ready(groups[j - 1][-1][0])
            run_hooks()
            emit_pv_ready(2 * NST)
            pv_e, pv_o = get_pv()
            if tail_pool is not None:
                norm_tail_fused(mt, lq, pv_e, pv_o, tail_pool)
                return None
            return (mt, lq, pv_e, pv_o)

        # ---- ancillary work queues ----
        kdone = [0]

        def kproj_m0_upto(scol, pool):
            while kdone[0] < min(scol, S):
                cw = min(512, S - kdone[0])
                ps = pool.tile([128, 512], f32, tag="pp", bufs=2,
                               name=f"psk0_{kdone[0]}")
                kproj_chunk(0, kdone[0], cw, ps)
                kdone[0] += cw

        # ======== passes ========
        # Norm of pass i is deferred into the start of pass i+1, so pool
        # transitions never wait the norm's DMA-latency chain and the next
        # pass's attention starts immediately.
        pend = {"n": None}

        def flush_norm():
            if pend["n"] is not None:
                norm_pass(*pend["n"])
                pend["n"] = None

        def run_pass(mt, lq, pname, pool, G, st_hook=None, tail_pool=None):
            r = attn_pass(mt, lq, pname, pool, G, st_hook, tail_pool,
                          pre_emit=flush_norm)
            if r is not None:
                pend["n"] = r

        # anc passes (G=2, exp N=1024, pp proj banks alive):
        #   (0,0): vproj + kproj-m0 pacing + qproj(0,1)
        #   (0,1): kproj-m1 chunks
        #   (0,2): remaining qproj chunks
        # pure passes (G=3, exp N=1536): (0,3) (1,0) (1,1) (1,2)
        # last pass (1,3): anc, out-proj hooks + fused norm tail
        with tc.tile_pool(name="pha", bufs=1, space="PSUM") as pha:
            # warm-up matmuls on already-memset SBUF while the input DMA
            # stream lands: ~4us of PE activity flips the HAM clock gate
            # to 2.4 GHz before the first real projections run
            wps = pha.tile([128, 256], f32, tag="pp", bufs=2, name="warmps")
            for w in range(24):
                nc.tensor.matmul(
                    out=wps, lhsT=V[0][:, 0:128], rhs=V[1][:, 0:256],
                    start=True, stop=True)
            ps = pha.tile([128, 512], f32, tag="pp", bufs=2, name="psq00")
            qproj_chunk(0, 0, ps)
            kproj_m0_upto(512, pha)

            def hook00(st):
                kproj_m0_upto((st + 4) * 128, pha)
                if st == 0:
                    for pre in range(2):
                        psv = pha.tile([128, 512], f32, tag="pp", bufs=2,
                                       name=f"psv{pre}")
                        vproj_tile(pre, psv)
                if st + 2 < NST:
                    psv = pha.tile([128, 512], f32, tag="pp", bufs=2,
                                   name=f"psv{st + 2}")
                    vproj_tile(st + 2, psv)
                if st == 30:
                    psq = pha.tile([128, 512], f32, tag="pp", bufs=2,
                                   name="psq01")
                    qproj_chunk(0, 1, psq)

            run_pass(0, 0, "p00", pha, 2, hook00)

            # (0,1) hooks: its successor's Q chunk first (a pass's own QT
            # quarter must be projected in an EARLIER pass — its first QK
            # groups are emitted before any of its own hooks run), then
            # the m1 K projection.
            ex01 = [("q", 0, 2)] + [
                ("k1", c * 512, min(512, S - c * 512))
                for c in range((S + 511) // 512)]

            def hook01(st):
                if ex01 and st % 3 == 0:
                    kind, a, b = ex01.pop(0)
                    ps = pha.tile([128, 512], f32, tag="pp", bufs=2,
                                  name=f"exA_{kind}_{a}_{b}")
                    if kind == "q":
                        qproj_chunk(a, b, ps)
                    else:
                        kproj_chunk(1, a, b, ps)

            run_pass(0, 1, "p01", pha, 2, hook01)
            while ex01:
                kind, a, b = ex01.pop(0)
                ps = pha.tile([128, 512], f32, tag="pp", bufs=2,
                              name=f"exAL_{a}_{b}")
                if kind == "q":
                    qproj_chunk(a, b, ps)
                else:
                    kproj_chunk(1, a, b, ps)

            ex02 = [(0, 3), (1, 0), (1, 1), (1, 2), (1, 3)]

            def hook02(st):
                if ex02 and st % 4 == 0:
                    m, qq = ex02.pop(0)
                    ps = pha.tile([128, 512], f32, tag="pp", bufs=2,
                                  name=f"exB_q{m}_{qq}")
                    qproj_chunk(m, qq, ps)

            run_pass(0, 2, "p02", pha, 2, hook02)
            assert not ex02, f"{len(ex02)} ancillary items left"

        # pure passes: G=3, exp N=1536
        with tc.tile_pool(name="phf", bufs=1, space="PSUM") as phf:
            for mt, lq, pname in [(0, 3, "p03"), (1, 0, "p10"),
                                  (1, 1, "p11"), (1, 2, "p12")]:
                run_pass(mt, lq, pname, phf, 3)

        # last pass: anc type; interleave out-proj for L quarters 0-1
        # (both pairs done for those), fused tail covers 2-3.
        with tc.tile_pool(name="phl", bufs=1, space="PSUM") as phl:
            outq = [(lc, nn) for lc in range(8) for nn in range(2)]

            def hook13(st):
                if outq:
                    lc, nn = outq.pop(0)
                    ps = phl.tile([128, 512], f32, tag="pp", bufs=2,
                                  name=f"po_{lc}_{nn}")
                    outproj_chunk(lc, nn, ps)

            run_pass(1, 3, "p13", phl, 2, hook13, tail_pool=phl)

    nself = (0 if os.environ.get("KSELFWAIT") == "keep"
             else _drop_self_waits(nc))
    ndrop = _dedupe_ldweights(nc)
    nsplit = _split_multiwaits(nc)
    if os.environ.get("KVERBOSE"):
        print(f"[kernel] dropped {ndrop} redundant ldweights, "
              f"{nself} self-waits, split {nsplit} multi-wait instrs")
    _BUILT[0] = nc
    return nc


def kernel(**inputs):
    global LAST_EXEC_TIME_NS
    from concourse import bass_utils

    x = np.asarray(inputs["x"], np.float32)
    ctx_mem = np.asarray(inputs["ctx_mem"], np.float32)
    pers_mem = np.asarray(inputs["pers_mem"], np.float32)
    Wq = np.asarray(inputs["Wq"], np.float32)
    Wk = np.asarray(inputs["Wk"], np.float32)
    Wv = np.asarray(inputs["Wv"], np.float32)
    Wo = np.asarray(inputs["Wo"], np.float32)
    bq = np.asarray(inputs["bq"], np.float32)
    bk = np.asarray(inputs["bk"], np.float32)
    bv = np.asarray(inputs["bv"], np.float32)
    bo = np.asarray(inputs["bo"], np.float32)

    nc = _build()

    extT_b = []
    for b in range(B):
        ext = np.concatenate([ctx_mem, pers_mem, x[b]], axis=0)  # [S, D]
        extT_b.append(np.ascontiguousarray(ext.T).astype(BF16))

    wq_bf = Wq.astype(BF16)
    wk_bf = Wk.astype(BF16)
    wv_bf = Wv.astype(BF16)
    wo_bf = Wo.astype(BF16)

    in_maps = []
    for c in range(NCORES):
        b, g = divmod(c, NCORES // B)
        cols = slice(g * HDPC, (g + 1) * HDPC)
        in_maps.append({
            "extT": extT_b[b],
            "wq": np.ascontiguousarray(wq_bf[:, cols]),
            "wk": np.ascontiguousarray(wk_bf[:, cols]),
            "wv": np.ascontiguousarray(wv_bf[:, cols]),
            "wo": np.ascontiguousarray(wo_bf[cols, :]),
            "bq": np.ascontiguousarray(bq[cols]),
            "bk": np.ascontiguousarray(bk[cols]),
        })

    res = bass_utils.run_bass_kernel_spmd(
        nc, in_maps, core_ids=list(range(NCORES)),
        trace=bool(os.environ.get("KPROF")),
    )
    LAST_EXEC_TIME_NS = res.exec_time_ns

    if os.environ.get("KDEBUG_CORES"):
        for c in range(NCORES):
            o = res.results[c]["out"]
            nanmask = ~np.isfinite(o)
            if nanmask.any():
                rows = np.where(nanmask.any(axis=1))[0]
                cols = np.where(nanmask.any(axis=0))[0]
                print(f"[core {c}] bad frac={nanmask.mean():.4f} "
                      f"rows [{rows.min()}..{rows.max()}] n={len(rows)} "
                      f"cols [{cols.min()}..{cols.max()}] n={len(cols)}")

    out = np.zeros((B, L, D), np.float32)
    for c in range(NCORES):
        b = c // (NCORES // B)
        out[b] += res.results[c]["out"]
    out += (bo + bv.astype(np.float32) @ Wo)[None, None, :]
    return out
